# revision 11
# baseline (speedup 1.0000x reference)
"""Trainium2 Bass kernel for nn_EnhancedTransformerEncoder (AGaLiTe-style gated
linear attention with oscillatory recurrences).

Strategy: data-parallel over batch (B=16 -> 2 per core x 8 cores), no
collectives. Per core the T=128 recurrence is computed in chunked
state-passing form (keys chunks of 64, values chunks of 32): per-channel
decays are handled with log-cumsum + exp (with per-chunk restarts), the
r-oscillator expansion and all F-contractions run on the tensor engine, and
termination resets are host-precomputed 0/1 masks folded into small
matmul-side tensors.
"""

import sys

sys.path.insert(0, "/opt/trn_rl_repo")

import numpy as np

import concourse.bass as bass
import concourse.bacc as bacc
import concourse.mybir as mybir
import concourse.tile as tile
from concourse.bass_utils import run_bass_kernel_spmd

T, B, D = 128, 16, 512
H, HD, ETA, R = 8, 64, 4, 8
F = ETA * HD
R9 = R + 1
CK, CV = 64, 64
JK, JV = T // CK, T // CV
EPS = 1e-6
NCORE, BL = 8, 2          # cores, local batches per core
OC = H * HD * 5 + 3 * H * ETA   # 2656 concatenated projection outputs

FP = mybir.dt.float32
AF = mybir.ActivationFunctionType
OP = mybir.AluOpType


# ----------------------------------------------------------------- host side

def _host_masks(terminations, tick, omegas):
    m = 1.0 - terminations.astype(np.float32)            # (T,B)
    ticks = np.arange(1, T + 1, dtype=np.float32)[:, None] + tick[None, :]
    cosx = np.ones((B, T, R9), np.float32)
    cosx[:, :, :R] = np.cos(ticks[:, :, None] * omegas[None, None, :]).transpose(1, 0, 2)
    cum = np.cumsum(terminations, axis=0)

    def masks(C):
        Jn = T // C
        mrow = np.zeros((B, T), np.float32)
        mcol = np.zeros((B, T), np.float32)
        mchunk = np.zeros((B, Jn), np.float32)
        Dm = np.zeros((B, Jn, C, C), np.float32)         # [s,t]
        for b in range(B):
            for j in range(Jn):
                c0, c1 = j * C, (j + 1) * C
                mm = m[c0:c1, b]
                mrow[b, c0:c1] = np.cumprod(mm)
                rev = np.cumprod(mm[::-1])[::-1]
                mc = np.ones(C, np.float32)
                mc[:-1] = rev[1:]
                mcol[b, c0:c1] = mc
                mchunk[b, j] = np.prod(mm)
                cu = cum[c0:c1, b]
                Dm[b, j] = ((cu[None, :] == cu[:, None]) &
                            (np.arange(C)[None, :] >= np.arange(C)[:, None])
                            ).astype(np.float32)
        return mrow, mcol, mchunk, Dm

    return m, cosx, masks


def host_prep(inputs):
    """Build every per-core numpy input from the full problem inputs."""
    term = np.asarray(inputs["terminations"])
    tick = np.asarray(inputs["tick"], np.float32)
    omegas = np.asarray(inputs["omegas"], np.float32)
    _, cosx, maskfn = _host_masks(term, tick, omegas)
    mrowK, mcolK, mchK, DmK = maskfn(CK)
    mrowV, mcolV, mchV, DmV = maskfn(CV)

    wcatT = np.ascontiguousarray(np.concatenate(
        [np.asarray(inputs[k], np.float32) for k in
         ("Wq", "Wk", "Wv", "Wbeta", "Wgamma", "Wp1", "Wp2", "Wp3")], axis=0).T)
    woutT = np.ascontiguousarray(np.asarray(inputs["Wout"], np.float32).T)
    boutv = np.asarray(inputs["bout"], np.float32).reshape(1, D)

    x = np.ascontiguousarray(np.asarray(inputs["inputs"], np.float32))

    # state init marshalling: kst0[b, p, rr, r9]  (channel c=(h,F): rr=c//128, p=c%128)
    kprev = np.asarray(inputs["tilde_k_prev"], np.float32)   # (B,R,H,F)
    sprev = np.asarray(inputs["s_prev"], np.float32)         # (B,H,F)
    kst0 = np.zeros((B, 128, 16, R9), np.float32)
    kch = kprev.transpose(0, 2, 3, 1).reshape(B, 2048, R)    # (B,(h,F),r)
    sch = sprev.reshape(B, 2048)
    kst0[:, :, :, :R] = kch.reshape(B, 16, 128, R).transpose(0, 2, 1, 3)
    kst0[:, :, :, R] = sch.reshape(B, 16, 128).transpose(0, 2, 1)
    vprev = np.asarray(inputs["tilde_v_prev"], np.float32)   # (B,R,H,HD)
    vch = vprev.transpose(0, 2, 3, 1).reshape(B, 512, R)     # (B,(h,hd),r)
    vst0 = vch.reshape(B, 4, 128, R).transpose(0, 2, 1, 3)   # (B,p,ct,r)

    ident = np.eye(128, dtype=np.float32)
    nbk = np.ones((128, T), np.float32)
    nbk[:, ::CK] = 0.0
    nbv = np.ones((128, T), np.float32)
    nbv[:, ::CV] = 0.0

    cosm = cosx * mcolK[:, :, None]                          # (B,T,9)
    cosmv = cosx[:, :, :R] * mcolV[:, :, None]               # (B,T,8)
    cosxTh = np.ascontiguousarray(cosx[:, :, :R].transpose(0, 2, 1))  # (B,8,T)

    cosxK = cosx.reshape(B, JK, CK, R9)                      # chunk-sliced, base-0
    mrowVrep = np.tile(mrowV[:, None, :], (1, 128, 1))       # (B,128,T)

    per_core = []
    for c in range(NCORE):
        bs = slice(c * BL, (c + 1) * BL)
        per_core.append({
            "xb": np.ascontiguousarray(x[:, bs, :]),
            "wcatT": wcatT, "woutT": woutT, "boutv": boutv,
            "ident": ident, "nbk": nbk, "nbv": nbv,
            "cosx": np.ascontiguousarray(cosx[bs]),
            "cosm": np.ascontiguousarray(cosm[bs]),
            "cosmv": np.ascontiguousarray(cosmv[bs]),
            "cosxT": np.ascontiguousarray(cosxTh[bs]),
            "mrowk": np.ascontiguousarray(mrowK[bs])[:, :, None],
            "mrowvrep": np.ascontiguousarray(mrowVrep[bs]),
            "cosxK": np.ascontiguousarray(cosxK[bs]),
            "dmk": np.ascontiguousarray(DmK[bs]),
            "dmv": np.ascontiguousarray(DmV[bs]),
            "mchk": np.ascontiguousarray(
                np.tile(mchK[bs][:, :, None, None], (1, 1, 128, 1))),
            "mchv": np.ascontiguousarray(
                np.tile(mchV[bs][:, :, None, None], (1, 1, 128, 1))),
            "kst0": np.ascontiguousarray(kst0[bs]),
            "vst0": np.ascontiguousarray(vst0[bs]),
        })
    return per_core


def host_finish(results, inputs):
    """Assemble full outputs from per-core result dicts."""
    out = np.zeros((T, B, D), np.float32)
    new_k = np.zeros((B, R, H, F), np.float32)
    new_v = np.zeros((B, R, H, HD), np.float32)
    new_s = np.zeros((B, H, F), np.float32)
    for c, res in enumerate(results):
        bs = slice(c * BL, (c + 1) * BL)
        out[:, bs, :] = res["outp"]
        kst = res["kstf"]                                    # (BL,128,16,9)
        kch = kst[:, :, :, :R].transpose(0, 2, 1, 3).reshape(BL, 2048, R)
        new_k[bs] = kch.reshape(BL, H, F, R).transpose(0, 3, 1, 2)
        new_s[bs] = kst[:, :, :, R].transpose(0, 2, 1).reshape(BL, H, F)
        vst = res["vstf"]                                    # (BL,128,4,8)
        vch = vst.transpose(0, 2, 1, 3).reshape(BL, 512, R)
        new_v[bs] = vch.reshape(BL, H, HD, R).transpose(0, 3, 1, 2)
    new_tick = np.asarray(inputs["tick"], np.float32) + np.float32(T)
    return out, new_k, new_v, new_s, new_tick


# --------------------------------------------------------------- device side

def build_device(tc, I, O):
    """Emit the Tile program. I/O are dicts of DRAM APs."""
    nc = tc.nc
    ex = lambda ap: ap  # readability

    with tc.tile_pool(name="wpool", bufs=1) as wp, \
         tc.tile_pool(name="cpool", bufs=1) as cp, \
         tc.tile_pool(name="big", bufs=1) as bp, \
         tc.tile_pool(name="med", bufs=1) as mp, \
         tc.tile_pool(name="small", bufs=2) as sp, \
         tc.tile_pool(name="psT", bufs=2, space="PSUM") as psT, \
         tc.tile_pool(name="ps", bufs=1, space="PSUM") as ps:

        # ---- constants / weights into SBUF
        wc = wp.tile([128, 4, OC], FP)
        for dt in range(4):
            nc.sync.dma_start(wc[:, dt, :], I["wcatT"][dt * 128:(dt + 1) * 128, :])
        wo = wp.tile([128, 4, D], FP)
        for dt in range(4):
            nc.sync.dma_start(wo[:, dt, :], I["woutT"][dt * 128:(dt + 1) * 128, :])
        ident = cp.tile([128, 128], FP)
        nc.sync.dma_start(ident[:], I["ident"])
        nbk = cp.tile([128, T], FP)
        nc.sync.dma_start(nbk[:], I["nbk"])
        nbv = cp.tile([128, T], FP)
        nc.sync.dma_start(nbv[:], I["nbv"])
        boutr = cp.tile([128, D], FP)
        nc.sync.dma_start(boutr[:], bass.AP(
            tensor=I["boutv"].tensor, offset=I["boutv"].offset,
            ap=[[0, 128]] + list(I["boutv"].ap[1:])))
        xT = cp.tile([128, BL, 4, 128], FP)
        for b in range(BL):
            for dt in range(4):
                nc.sync.dma_start(
                    xT[:, b, dt, :],
                    I["xb"][:, b, dt * 128:(dt + 1) * 128].rearrange("t d -> d t"))

        epsc = cp.tile([128, 1], FP)
        nc.vector.memset(epsc[:], EPS)

        for b in range(BL):
            # ---------- projections (T-layout out: tokens on partitions)
            qs = mp.tile([128, 512], FP, tag="qs")
            ks = mp.tile([128, 512], FP, tag="ks")
            vs = mp.tile([128, 512], FP, tag="vs")
            bts = mp.tile([128, 512], FP, tag="bts")
            gms = mp.tile([128, 512], FP, tag="gms")
            pps = mp.tile([128, 96], FP, tag="pps")
            evac = [(qs, AF.Relu), (ks, AF.Relu), (vs, AF.Copy),
                    (bts, AF.Sigmoid), (gms, AF.Sigmoid)]
            for oc in range(6):
                o0 = oc * 512
                ow = 512 if oc < 5 else 96
                pj = ps.tile([128, 512], FP, tag="pjpo")
                for dt in range(4):
                    nc.tensor.matmul(pj[:, :ow], xT[:, b, dt, :],
                                     wc[:, dt, o0:o0 + ow],
                                     start=(dt == 0), stop=(dt == 3))
                if oc < 5:
                    dst, fn = evac[oc]
                    nc.scalar.activation(dst[:], pj[:, :ow], fn)
                else:
                    nc.scalar.activation(pps[:, 0:64], pj[:, 0:64], AF.Relu)
                    nc.scalar.activation(pps[:, 64:96], pj[:, 64:96], AF.Sigmoid)

            p1s, p2s, p3s = pps[:, 0:32], pps[:, 32:64], pps[:, 64:96]

            # ---------- T-layout elementwise
            kgs = mp.tile([128, 512], FP, tag="kgs")
            nc.vector.tensor_tensor(kgs[:], ks[:], gms[:], OP.mult)
            gvs = mp.tile([128, 512], FP, tag="gvs")
            nc.vector.tensor_tensor(gvs[:], vs[:], bts[:], OP.mult)
            pgs = sp.tile([128, 32], FP, tag="pgs")
            nc.vector.tensor_tensor(pgs[:], p1s, p3s, OP.mult)

            def outer(dst, pside, xside):
                # dst[(h,e,hd)] = pside[(h,e)] * xside[(h,hd)]
                pa = pside.rearrange("p (h e) -> p h e", h=H).unsqueeze(3) \
                          .to_broadcast([128, H, ETA, HD])
                xa = xside.rearrange("p (h d) -> p h d", h=H).unsqueeze(2) \
                          .to_broadcast([128, H, ETA, HD])
                da = dst.rearrange("p (h e d) -> p h e d", h=H, e=ETA)
                nc.vector.tensor_tensor(da, pa, xa, OP.mult)

            phi = bp.tile([128, 2048], FP, tag="phi")
            outer(phi[:], p2s, qs[:])
            gkt = bp.tile([128, 2048], FP, tag="gkt")
            outer(gkt[:], pgs[:], kgs[:])
            gfs = bp.tile([128, 2048], FP, tag="gfs")
            outer(gfs[:], p3s, gms[:])

            # ---------- keys-side decay chain (F-layout)
            loggT = bp.tile([128, 16, 128], FP, tag="loggT")
            for rr in range(16):
                tp = psT.tile([128, 512], FP, tag="tpA")
                nc.tensor.transpose(tp[:, 0:128], gfs[:, rr * 128:(rr + 1) * 128],
                                    ident[:])
                nc.scalar.activation(loggT[:, rr, :], tp[:, 0:128], AF.Ln,
                                     bias=1.0, scale=-1.0)
            LT = bp.tile([128, 16, 128], FP, tag="LT")
            for rr in range(16):
                nc.vector.tensor_tensor_scan(
                    LT[:, rr, :], nbk[:], loggT[:, rr, :], 0.0, OP.mult, OP.add)
            lamT = bp.tile([128, 2048], FP, tag="lamT")
            nc.scalar.activation(lamT[:], LT[:].rearrange("p a b -> p (a b)"), AF.Exp)
            lamiT = bp.tile([128, 2048], FP, tag="lamiT")
            nc.scalar.activation(lamiT[:], LT[:].rearrange("p a b -> p (a b)"),
                                 AF.Exp, scale=-1.0)

            wT = bp.tile([128, 16, 128], FP, tag="wT")
            uFT = bp.tile([128, 16, 128], FP, tag="uFT")
            for src, dst, lam in ((phi, wT, lamT), (gkt, uFT, lamiT)):
                for q4 in range(4):
                    tp4 = psT.tile([128, 512], FP, tag="tpA")
                    for i in range(4):
                        rr = q4 * 4 + i
                        nc.tensor.transpose(
                            tp4[:, i * 128:(i + 1) * 128],
                            src[:, rr * 128:(rr + 1) * 128], ident[:])
                    nc.vector.tensor_tensor(
                        dst[:, q4 * 4:(q4 + 1) * 4, :]
                            .rearrange("p a b -> p (a b)"),
                        tp4[:], lam[:, q4 * 512:(q4 + 1) * 512], OP.mult)
            uT = bp.tile([128, 2048], FP, tag="uT")
            for q4 in range(4):
                tp4 = psT.tile([128, 512], FP, tag="tpA")
                for i in range(4):
                    rr = q4 * 4 + i
                    nc.tensor.transpose(
                        tp4[:, i * 128:(i + 1) * 128],
                        uFT[:, rr, :], ident[:])
                nc.scalar.activation(uT[:, q4 * 512:(q4 + 1) * 512], tp4[:], AF.Copy)

            # ---------- values-side decay chain
            logbT = mp.tile([128, 4, 128], FP, tag="logbT")
            for ct in range(4):
                tp = psT.tile([128, 512], FP, tag="tpA")
                nc.tensor.transpose(tp[:, 0:128], bts[:, ct * 128:(ct + 1) * 128],
                                    ident[:])
                nc.scalar.activation(logbT[:, ct, :], tp[:, 0:128], AF.Ln,
                                     bias=1.0, scale=-1.0)
            LbT = mp.tile([128, 4, 128], FP, tag="LbT")
            for ct in range(4):
                nc.vector.tensor_tensor_scan(
                    LbT[:, ct, :], nbv[:], logbT[:, ct, :], 0.0, OP.mult, OP.add)
            lamb = mp.tile([128, 512], FP, tag="lamb")
            nc.scalar.activation(lamb[:], LbT[:].rearrange("p a b -> p (a b)"), AF.Exp)
            lambi = mp.tile([128, 512], FP, tag="lambi")
            nc.scalar.activation(lambi[:], LbT[:].rearrange("p a b -> p (a b)"),
                                 AF.Exp, scale=-1.0)
            # transpose lamb, lambi back to T-layout
            lambTT = mp.tile([128, 512], FP, tag="lambTT")
            lambiTT = mp.tile([128, 512], FP, tag="lambiTT")
            for src, dst in ((lamb, lambTT), (lambi, lambiTT)):
                tp4 = psT.tile([128, 512], FP, tag="tpA")
                for ct in range(4):
                    nc.tensor.transpose(
                        tp4[:, ct * 128:(ct + 1) * 128],
                        src.rearrange("p (a b) -> p a b", a=4)[:, ct, :], ident[:])
                nc.scalar.activation(dst[:], tp4[:], AF.Copy)
            uv = mp.tile([128, 512], FP, tag="uv")
            nc.vector.tensor_tensor(uv[:], gvs[:], lambiTT[:], OP.mult)

            # ---------- keys chunk loop
            kst = mp.tile([128, 16, R9], FP, tag="kst")
            nc.sync.dma_start(kst[:], I["kst0"][b])
            cosx_b = sp.tile([128, R9], FP, tag="cosx_b")
            nc.sync.dma_start(cosx_b[:], I["cosx"][b])
            cosm_b = sp.tile([128, R9], FP, tag="cosm_b")
            nc.sync.dma_start(cosm_b[:], I["cosm"][b])
            mrowk_b = sp.tile([128, 1], FP, tag="mrowk_b")
            nc.sync.dma_start(mrowk_b[:], I["mrowk"][b])

            kdq = mp.tile([128, 72], FP, tag="kdq")
            for j in range(JK):
                t0 = j * CK
                # (a) intra P~T[s, h, t] = sum_F uF[s] w[t], h on free axis
                pP = ps.tile([CK, H, CK], FP, tag="pPc2")
                for kk in range(16):
                    h = kk // 2
                    nc.tensor.matmul(
                        pP[0:CK, h, :],
                        uFT[:, kk, t0:t0 + CK], wT[:, kk, t0:t0 + CK],
                        start=(kk % 2 == 0), stop=(kk % 2 == 1))
                dmk_j = sp.tile([CK, CK], FP, tag="dmk_j")
                nc.sync.dma_start(dmk_j[:], I["dmk"][b, j])
                ptm = mp.tile([CK, H, CK], FP, tag="ptm")
                nc.vector.tensor_tensor(
                    ptm[:, :, :], pP[:, :, :],
                    dmk_j[:].unsqueeze(1).to_broadcast([CK, H, CK]),
                    OP.mult)
                cosxK_j = sp.tile([CK, R9], FP, tag="cosxK_j")
                nc.sync.dma_start(cosxK_j[:], I["cosxK"][b, j])
                # (b) state term + intra cos-matmul into separate psums
                pS = ps.tile([128, 72], FP, tag="pSBv")
                for kk in range(16):
                    nc.tensor.matmul(
                        pS[t0:t0 + CK, (kk // 2) * 9:(kk // 2) * 9 + 9],
                        wT[:, kk, t0:t0 + CK], kst[:, kk, :],
                        start=(kk % 2 == 0), stop=(kk % 2 == 1))
                pI = ps.tile([128, 72], FP, tag="pIvT")
                for h in range(H):
                    nc.tensor.matmul(
                        pI[t0:t0 + CK, h * 9:h * 9 + 9],
                        ptm[0:CK, h, :],
                        cosxK_j[:, :],
                        start=True, stop=True)
                tmpi = sp.tile([128, 72], FP, tag="tmpi")
                nc.vector.tensor_copy(tmpi[t0:t0 + CK, :], pI[t0:t0 + CK, :])
                nc.vector.scalar_tensor_tensor(
                    kdq[t0:t0 + CK, :], pS[t0:t0 + CK, :],
                    mrowk_b[t0:t0 + CK, :], tmpi[t0:t0 + CK, :],
                    OP.mult, OP.add)
                # boundary state update
                pB = ps.tile([128, 16, 9], FP, tag="pBkT")
                for kk in range(16):
                    nc.tensor.matmul(
                        pB[:, kk, :],
                        uT[t0:t0 + CK, kk * 128:(kk + 1) * 128],
                        cosm_b[t0:t0 + CK, :], start=True, stop=True)
                lame = sp.tile([128, 16], FP, tag="lame")
                mchk_j = sp.tile([128, 1], FP, tag="mchk_j")
                nc.sync.dma_start(mchk_j[:], I["mchk"][b, j])
                nc.vector.tensor_copy(
                    lame[:], lamT.rearrange("p (a b) -> p a b", a=16)[:, :, t0 + CK - 1])
                lamem = sp.tile([128, 16], FP, tag="lamem")
                nc.vector.tensor_tensor(
                    lamem[:], lame[:], mchk_j[:].to_broadcast([128, 16]), OP.mult)
                nc.vector.tensor_tensor(
                    kst[:], kst[:],
                    lamem[:].unsqueeze(2).to_broadcast([128, 16, R9]), OP.mult)
                tmpB = sp.tile([128, 16, 9], FP, tag="tmpB")
                nc.vector.tensor_tensor(
                    tmpB[:], pB[:],
                    lame[:].unsqueeze(2).to_broadcast([128, 16, 9]), OP.mult)
                nc.vector.tensor_tensor(
                    kst[:, :, :], kst[:, :, :],
                    tmpB[:], OP.add)

            nc.sync.dma_start(O["kstf"][b], kst[:])

            # ---------- values side: kdqT[r, h, t] per-h transposes (base 0)
            kdqT = sp.tile([8, H, 128], FP, tag="kdqT")
            for g in range(2):
                kps = ps.tile([8, 4, 128], FP, tag="pBkT")
                for i in range(4):
                    h = g * 4 + i
                    nc.tensor.transpose(kps[0:8, i, :], kdq[:, h * 9:h * 9 + 8],
                                        ident[:])
                nc.scalar.activation(kdqT[:, g * 4:g * 4 + 4, :], kps[:], AF.Copy)
            mrv = sp.tile([8, 128], FP, tag="mrv")
            nc.sync.dma_start(mrv[:], I["mrowvrep"][b, 0:8, :])
            kdqTm = sp.tile([8, H, 128], FP, tag="kdqTm")
            nc.vector.tensor_tensor(
                kdqTm[:, :, :], kdqT[:, :, :],
                mrv[:].unsqueeze(1).to_broadcast([8, H, 128]), OP.mult)

            cosxT_b = sp.tile([8, 128], FP, tag="cosxT_b")
            nc.sync.dma_start(cosxT_b[:], I["cosxT"][b])
            cosmv_b = sp.tile([128, 8], FP, tag="cosmv_b")
            nc.sync.dma_start(cosmv_b[:], I["cosmv"][b])

            vst = sp.tile([128, 4, 8], FP, tag="vst")
            nc.sync.dma_start(vst[:], I["vst0"][b])
            pkv = ps.tile([128, 512], FP, tag="pkv")
            for jv in range(JV):
                t0 = jv * CV
                # c2^T[s, h, t]: out rows at base t0, h on free
                pc2 = ps.tile([128, H, CV], FP, tag="pPc2")
                for h in range(H):
                    nc.tensor.matmul(
                        pc2[t0:t0 + CV, h, :],
                        cosxT_b[:, t0:t0 + CV],
                        kdqT[0:8, h, t0:t0 + CV],
                        start=True, stop=True)
                dmv_j = sp.tile([CV, CV], FP, tag="dmv_j")
                nc.sync.dma_start(dmv_j[:], I["dmv"][b, jv])
                c2m = sp.tile([128, H, CV], FP, tag="c2m")
                nc.vector.tensor_tensor(
                    c2m[t0:t0 + CV, :, :], pc2[t0:t0 + CV, :, :],
                    dmv_j[:].unsqueeze(1).to_broadcast([CV, H, CV]), OP.mult)
                # VstT[r, ct, ch] for the state term (base 0)
                pvT = ps.tile([8, 4, 128], FP, tag="pIvT")
                for ct in range(4):
                    nc.tensor.transpose(pvT[0:8, ct, :], vst[:, ct, :], ident[:])
                vstT = sp.tile([8, 4, 128], FP, tag="vstT")
                nc.scalar.activation(vstT[:], pvT[:], AF.Copy)
                for h in range(H):
                    nc.tensor.matmul(
                        pkv[t0:t0 + CV, h * 64:h * 64 + 64],
                        c2m[t0:t0 + CV, h, :],
                        uv[t0:t0 + CV, h * 64:h * 64 + 64],
                        start=True, stop=False)
                    nc.tensor.matmul(
                        pkv[t0:t0 + CV, h * 64:h * 64 + 64],
                        kdqTm[0:8, h, t0:t0 + CV],
                        vstT[0:8, h // 2, (h % 2) * 64:(h % 2) * 64 + 64],
                        start=False, stop=True)
                # boundary update
                pBv = ps.tile([128, 4, 8], FP, tag="pSBv")
                for ct in range(4):
                    nc.tensor.matmul(
                        pBv[:, ct, :],
                        uv[t0:t0 + CV, ct * 128:(ct + 1) * 128],
                        cosmv_b[t0:t0 + CV, :], start=True, stop=True)
                lbe = sp.tile([128, 4], FP, tag="lbe")
                nc.vector.tensor_copy(
                    lbe[:], lamb.rearrange("p (a b) -> p a b", a=4)[:, :, t0 + CV - 1])
                mchv_j = sp.tile([128, 1], FP, tag="mchv_j")
                nc.sync.dma_start(mchv_j[:], I["mchv"][b, jv])
                lbem = sp.tile([128, 4], FP, tag="lbem")
                nc.vector.tensor_tensor(
                    lbem[:], lbe[:], mchv_j[:].to_broadcast([128, 4]), OP.mult)
                nc.vector.tensor_tensor(
                    vst[:], vst[:],
                    lbem[:].unsqueeze(2).to_broadcast([128, 4, 8]), OP.mult)
                tmpBv = sp.tile([128, 4, 8], FP, tag="tmpBv")
                nc.vector.tensor_tensor(
                    tmpBv[:], pBv[:],
                    lbe[:].unsqueeze(2).to_broadcast([128, 4, 8]), OP.mult)
                nc.vector.tensor_tensor(vst[:], vst[:], tmpBv[:], OP.add)

            nc.sync.dma_start(O["vstf"][b], vst[:])

            kvs = mp.tile([128, 512], FP, tag="kvs")
            nc.vector.tensor_tensor(kvs[:], pkv[:], lambTT[:], OP.mult)

            # ---------- attn scaling + output projection
            den = sp.tile([128, 8], FP, tag="den")
            nc.vector.scalar_tensor_tensor(
                den[:], kdq.rearrange("p (h r) -> p h r", h=H)[:, :, R],
                float(2 * R), epsc[:].to_broadcast([128, 8]), OP.mult, OP.add)
            rec = sp.tile([128, 8], FP, tag="rec")
            nc.vector.reciprocal(rec[:], den[:])
            attn = mp.tile([128, 512], FP, tag="attn")
            nc.vector.tensor_tensor(
                attn.rearrange("p (h d) -> p h d", h=H),
                kvs.rearrange("p (h d) -> p h d", h=H),
                rec[:].unsqueeze(2).to_broadcast([128, H, HD]), OP.mult)
            aT4 = psT.tile([128, 512], FP, tag="tpA")
            for ct in range(4):
                nc.tensor.transpose(
                    aT4[:, ct * 128:(ct + 1) * 128],
                    attn[:, ct * 128:(ct + 1) * 128], ident[:])
            attnT = mp.tile([128, 4, 128], FP, tag="attnT")
            nc.scalar.activation(
                attnT[:].rearrange("p a b -> p (a b)"), aT4[:], AF.Copy)
            po = ps.tile([128, 512], FP, tag="pjpo")
            for ct in range(4):
                nc.tensor.matmul(po[:], attnT[:, ct, :], wo[:, ct, :],
                                 start=(ct == 0), stop=(ct == 3))
            outs = mp.tile([128, 512], FP, tag="outs")
            nc.vector.tensor_tensor(outs[:], po[:], boutr[:], OP.add)
            nc.sync.dma_start(O["outp"][:, b, :], outs[:])


# ------------------------------------------------------------------ runners

_CACHE = {}
LAST = {}


def _build_nc():
    nc = bacc.Bacc("TRN2", target_bir_lowering=False, debug=False,
                   enable_asserts=False, num_devices=NCORE)
    ins = {
        "xb": (T, BL, D), "wcatT": (512, OC), "woutT": (D, D), "boutv": (1, D),
        "ident": (128, 128), "nbk": (128, T), "nbv": (128, T),
        "cosx": (BL, T, R9), "cosm": (BL, T, R9), "cosmv": (BL, T, R),
        "cosxT": (BL, R, T), "mrowk": (BL, T, 1), "mrowvrep": (BL, 128, T),
        "cosxK": (BL, JK, CK, R9),
        "dmk": (BL, JK, CK, CK), "dmv": (BL, JV, CV, CV),
        "mchk": (BL, JK, 128, 1), "mchv": (BL, JV, 128, 1),
        "kst0": (BL, 128, 16, R9), "vst0": (BL, 128, 4, R),
    }
    outs = {"outp": (T, BL, D), "kstf": (BL, 128, 16, R9),
            "vstf": (BL, 128, 4, R)}
    I = {k: nc.dram_tensor(k, list(s), FP, kind="ExternalInput").ap()
         for k, s in ins.items()}
    O = {k: nc.dram_tensor(k, list(s), FP, kind="ExternalOutput").ap()
         for k, s in outs.items()}
    with tile.TileContext(nc) as tc:
        build_device(tc, I, O)
    nc.compile()
    return nc


def kernel(**inputs):
    per_core = host_prep(inputs)
    if "nc" not in _CACHE:
        _CACHE["nc"] = _build_nc()
    nc = _CACHE["nc"]
    res = run_bass_kernel_spmd(nc, per_core, core_ids=list(range(NCORE)))
    LAST["exec_time_ns"] = res.exec_time_ns
    LAST["trace"] = res.instructions_and_trace
    return host_finish(res.results, inputs)


if __name__ == "__main__":
    sys.path.insert(0, "/root/problem")
    import reference as Rf
    inp = Rf.setup_inputs()
    exp = Rf.reference(**inp)
    got = kernel(**{k: np.asarray(v) for k, v in inp.items()})
    for n, e, g in zip(["output", "new_k", "new_v", "new_s", "new_tick"], exp, got):
        e = np.asarray(e)
        err = np.abs(e - np.asarray(g)).max() / max(1e-6, np.abs(e).max())
        print(f"{n}: relmax-err {err:.3e}")
